# revision 36
# baseline (speedup 1.0000x reference)
"""Multi-head attention (no softmax) on 8 TRN2 NeuronCores.

Problem: x[2,2048,1024], per-head Wq/Wk/Wv[16,64,1024] + biases.
    q = einsum('bsd,hed->bhse', x, Wq) + bq   (same for k, v)
    out = ((q @ k^T) * E^-0.5) @ v, heads concatenated on feature dim.

Key algebraic fact: there is NO softmax, so
    (q k^T * norm) v = q @ (norm * (k^T v))
which collapses the O(S^2) attention into a 64x64 (per head) matmul.

Sharding: 2D tensor-parallel over (batch, head-quad): core c owns batch
c//4 and heads 4*(c%4) .. 4*(c%4)+3.  Each core reads only its batch's
half of x (8.4MB), and processes its 4 heads as two packed head-pairs
(feature groups g=0,1 of 128).

Per core (all matmuls fp32r = fp32 storage, ~fp22 multiply, fp32 psum):
  phase 1: project QT/KT/VT[g] = W[g] @ x_b^T in [feat(128), seq(2048)]
           layout (N=512 moving dim -> full PE speed).  norm is folded
           into Wq/bq on the host; biases are per-partition adds fused
           into the PSUM->SBUF copies.  j-outer / g-inner so each DMA'd
           x chunk feeds both groups (the x stream is HBM-bound).
  phase 2: trailing one seq-chunk behind phase 1: PE-transpose K/V tiles
           to [seq, feat] and accumulate M_g = K^T V [128,128] in PSUM
           over 16 seq-chunks (M matmul software-pipelined one step
           behind its transposes); copy the two diagonal 64x64 head
           blocks into a zeroed SBUF tile (cross-head blocks of M are
           garbage and must be dropped).
  phase 3: outT[g][:, s-chunk] = M_g(blockdiag).T @ QT[g][:, s-chunk].
Host gathers: out[c//4, s, (c%4)*256+g*128 +:128] = outT_c[g][:, s].T

Scheduling notes (why the DMA/program order looks the way it does):
  - one HWDGE dispatch is ~650ns regardless of size; the Sync queue
    carries only wq0-chunks interleaved with x(:,0), then wk0/wv0, then
    the rest of x.  Ident, biases and g=1 weights ride the ACT HWDGE
    queue (needed later; keeping them off the x stream matters).
  - dummy bf16 matmuls at kernel start hold the PE's HAM clock gate
    open (cold PE runs at 1.2GHz for its first ~3.4us) while the first
    x tiles land.
"""

import numpy as np

import concourse.bacc as bacc
import concourse.tile as tile
import concourse.mybir as mybir
from concourse import bass2jax

B, S, D, H = 2, 2048, 1024, 16
E = 64          # head dim
NCORES = 8
NB = NCORES // B            # cores per batch (4)
HL = H // NB                # heads per core (4)
NG = 2                      # feature groups per core (head pairs)
EP = HL * E // NG           # packed feature dim per group (128)
P = 128                     # partitions
DC = D // P                 # d chunks (8)
SC = 512                    # seq chunk for N=512 matmuls
NSC = S // SC               # 4 seq chunks
TC = S // P                 # 16 transpose chunks per group
NORM = float(E) ** -0.5

F32 = mybir.dt.float32
F32R = mybir.dt.float32r

_compiled = None


def _build():
    nc = bacc.Bacc("TRN2", target_bir_lowering=False, debug=False)

    x_d = nc.dram_tensor("x", [DC, NSC, P, SC], F32R, kind="ExternalInput").ap()
    w_d = {}
    for g in range(NG):
        for wn in ("wq", "wk", "wv"):
            w_d[wn, g] = nc.dram_tensor(
                f"{wn}{g}", [P, DC, P], F32R, kind="ExternalInput").ap()
    b_d = {}
    for g in range(NG):
        for bn in ("bq", "bk", "bv"):
            b_d[bn, g] = nc.dram_tensor(
                f"{bn}{g}", [P, 1], F32, kind="ExternalInput").ap()
    id_d = nc.dram_tensor("ident", [P, P], F32R, kind="ExternalInput").ap()
    out_d = nc.dram_tensor("outT", [NG, P, S], F32, kind="ExternalOutput").ap()

    with tile.TileContext(nc) as tc:
        with (
            tc.tile_pool(name="consts", bufs=1) as consts,
            tc.tile_pool(name="xs", bufs=16) as xs_pool,
            tc.tile_pool(name="qkv", bufs=1) as qkv_pool,
            tc.tile_pool(name="kv", bufs=8) as kv_pool,
            tc.tile_pool(name="mt", bufs=1) as mt_pool,
            tc.tile_pool(name="ot", bufs=8) as ot_pool,
            tc.tile_pool(name="pproj", bufs=6, space="PSUM") as pproj,
            tc.tile_pool(name="pm", bufs=2, space="PSUM") as pm,
        ):
            # ---- input DMAs (order matters; see scheduling notes above)
            xs = {}

            def load_x(i, jp):
                # one 512KB DMA per (d-chunk, seq-chunk-pair): the HWDGE
                # queue costs ~600ns dispatch per DMA regardless of size,
                # so fewer/larger transfers shorten the x stream
                t = xs_pool.tile([P, 2, SC], F32R, tag="xs", name=f"x_{i}_{jp}")
                nc.sync.dma_start(
                    t[:], x_d[i, 2 * jp:2 * jp + 2].rearrange("a p s -> p a s"))
                xs[i, jp] = t

            w_tiles, b_tiles = {}, {}

            def load_w(wn, g):
                wt = consts.tile([P, DC, P], F32R, tag=f"{wn}{g}", name=f"{wn}{g}_t")
                nc.sync.dma_start(wt[:], w_d[wn, g][:])
                w_tiles[wn, g] = wt

            # interleave the first weight chunks with the first x tiles so
            # the first accumulation group can start after ~0.5MB of DMA
            wq0 = consts.tile([P, DC, P], F32R, tag="wq0", name="wq0_t")
            w_tiles["wq", 0] = wq0
            for i in range(DC):
                nc.sync.dma_start(wq0[:, i, :], w_d["wq", 0][:, i, :])
                load_x(i, 0)
            load_w("wk", 0)
            load_w("wv", 0)
            # Biases, ident and g=1 weights ride the ACT HWDGE queue so the
            # Sync queue stays a pure x stream (it is HBM-bound; every DMA
            # inserted into it delays the x tiles phase 1 is waiting for).
            # Order: ident + g=1 weights are needed within ~10-16us (the
            # j-interleaved schedule uses both groups' weights early).
            ident = consts.tile([P, P], F32R, tag="ident")
            nc.scalar.dma_start(ident[:], id_d[:])
            for wn in ("wq", "wk", "wv"):
                wt = consts.tile([P, DC, P], F32R, tag=f"{wn}1", name=f"{wn}1_t")
                nc.scalar.dma_start(wt[:], w_d[wn, 1][:])
                w_tiles[wn, 1] = wt
            for bn in ("bq", "bk", "bv"):
                for g in range(NG):
                    bt = consts.tile([P, 1], F32, tag=f"{bn}{g}", name=f"{bn}{g}_t")
                    nc.scalar.dma_start(bt[:], b_d[bn, g][:])
                    b_tiles[bn, g] = bt
            for jp in range(1, NSC // 2):
                for i in range(DC):
                    load_x(i, jp)

            # ---- PE clock warmup: the HAM gate holds the PE at 1.2GHz for
            # its first ~3.4us of sustained activity, and the PE is idle
            # until the first x tiles land anyway.  Burn that window with
            # dummy matmuls on a zeroed scratch tile so the real matmuls
            # start at 2.4GHz.  Results go to a psum tile nobody reads.
            warm_in = consts.tile([P, 256], mybir.dt.bfloat16, tag="warm")
            nc.gpsimd.memset(warm_in[:], 0.0)
            warm_ps = pproj.tile([P, 256], F32, tag="proj", name="warm_ps")
            for _ in range(10):
                nc.tensor.matmul(warm_ps[:], warm_in[:, 0:P], warm_in[:],
                                 start=True, stop=True)

            # ---- phases 1+2, transposes trailing one seq chunk
            big = {}
            for g in range(NG):
                for tn in ("q", "k", "v"):
                    big[tn, g] = qkv_pool.tile([P, S], F32R, tag=f"{tn}t{g}",
                                               name=f"{tn}t{g}")
            m_psum = {}
            pending = {g: None for g in range(NG)}  # per-g pipelined M matmul

            def transpose_chunk(g, j):
                for tt in range(SC // P):
                    t = j * (SC // P) + tt
                    sl = slice(t * P, (t + 1) * P)
                    ktp = pproj.tile([P, P], F32R, tag="proj",
                                     name=f"ktp_{g}_{t}")
                    nc.tensor.transpose(ktp[:], big["k", g][:, sl], ident[:])
                    k_sb = kv_pool.tile([P, P], F32R, tag="k_sb",
                                        name=f"k_sb_{g}_{t}")
                    nc.scalar.copy(k_sb[:], ktp[:])
                    vtp = pproj.tile([P, P], F32R, tag="proj",
                                     name=f"vtp_{g}_{t}")
                    nc.tensor.transpose(vtp[:], big["v", g][:, sl], ident[:])
                    v_sb = kv_pool.tile([P, P], F32R, tag="v_sb",
                                        name=f"v_sb_{g}_{t}")
                    nc.vector.tensor_copy(v_sb[:], vtp[:])
                    if pending[g] is not None:
                        p = pending[g]
                        nc.tensor.matmul(
                            m_psum[g][:], p[0][:], p[1][:],
                            start=(p[2] == 0), stop=(p[2] == TC - 1),
                            skip_group_check=True)
                    pending[g] = (k_sb, v_sb, t)

            def proj_group(tn, wn, bn, g, j, filler=0):
                ps = pproj.tile([P, SC], F32, tag="proj",
                                name=f"ps_{tn}{g}_{j}")
                for i in range(DC):
                    nc.tensor.matmul(
                        ps[:], w_tiles[wn, g][:, i, :], xs[i, j // 2][:, j % 2, :],
                        start=(i == 0), stop=(i == DC - 1),
                        skip_group_check=bool(filler),
                    )
                    # dummy matmuls between the accumulation steps: during
                    # the first chunk the x tiles land one DMA (~650ns)
                    # apart, slower than the PE consumes them, so the PE is
                    # provably idle here; the filler keeps the HAM clock
                    # gate open and soaks up the wait.
                    if filler and i < DC - 1:
                        for _ in range(filler):
                            nc.tensor.matmul(warm_ps[:], warm_in[:, 0:P],
                                             warm_in[:], start=True, stop=True,
                                             skip_group_check=True)
                sl = big[tn, g][:, j * SC:(j + 1) * SC]
                if tn == "v":
                    nc.scalar.activation(
                        sl, ps[:], mybir.ActivationFunctionType.Identity,
                        bias=b_tiles[bn, g][:])
                else:
                    nc.vector.tensor_scalar_add(sl, ps[:], b_tiles[bn, g][:])

            # j-outer, tensor-outer, g-inner: each x chunk feeds BOTH groups'
            # projections (48 matmuls per 2MB of DMA), so the HBM-bound x
            # stream never starves the PE after the first chunk, and each
            # weight tensor is first needed in the order the two DMA queues
            # deliver them (wq0 | wq1, wk1, wv1 (ACT) | wk0, wv0 (sync)).
            # Transposes trail one chunk behind.
            for g in range(NG):
                m_psum[g] = pm.tile([P, P], F32, tag="m", name=f"mps_{g}")
            for j in range(NSC):
                for g in range(NG):
                    for tn, wn, bn in (("q", "wq", "bq"), ("k", "wk", "bk"),
                                       ("v", "wv", "bv")):
                        proj_group(tn, wn, bn, g, j)
                for g in range(NG):
                    if j > 0:
                        transpose_chunk(g, j - 1)
            for g in range(NG):
                transpose_chunk(g, NSC - 1)
                p = pending[g]
                nc.tensor.matmul(m_psum[g][:], p[0][:], p[1][:],
                                 start=(p[2] == 0), stop=True,
                                 skip_group_check=True)
                mt = mt_pool.tile([P, P], F32R, tag=f"mt{g}", name=f"mt_{g}")
                # zero-fill without InstMemset (walrus rejects f32r memset)
                nc.vector.tensor_scalar_mul(mt[:], ident[:], 0.0)
                nc.vector.tensor_copy(mt[0:E, 0:E], m_psum[g][0:E, 0:E])
                nc.vector.tensor_copy(mt[E:P, E:P], m_psum[g][E:P, E:P])

                # phase 3 for this group
                for j in range(NSC):
                    sl = slice(j * SC, (j + 1) * SC)
                    ps = pproj.tile([P, SC], F32, tag="proj", name=f"ops_{g}_{j}")
                    nc.tensor.matmul(ps[:], mt[:], big["q", g][:, sl],
                                     start=True, stop=True)
                    ot = ot_pool.tile([P, SC], F32, tag="ot", name=f"ot_{g}_{j}")
                    nc.vector.tensor_copy(ot[:], ps[:])
                    nc.sync.dma_start(out_d[g, :, sl], ot[:])

    nc.compile()
    return nc


def _prep_inputs(x, Wq, Wk, Wv, bq, bk, bv):
    """Host-side shard + layout prep. Returns per-core input maps."""
    x_tiles_b = []
    for b in range(B):
        xf = np.ascontiguousarray(x[b].T)                   # [D, S]
        x_tiles_b.append(np.ascontiguousarray(
            xf.reshape(DC, P, NSC, SC).transpose(0, 2, 1, 3)))

    def wlayout(w):                                         # [P, D] -> [P, DC, P]
        return np.ascontiguousarray(w.T.reshape(DC, P, P).transpose(1, 0, 2))

    in_maps = []
    for c in range(NCORES):
        b = c // NB
        q0 = HL * (c % NB)                                  # first head of core
        m = {"x": x_tiles_b[b], "ident": np.eye(P, dtype=np.float32)}
        for g in range(NG):
            hs = slice(q0 + 2 * g, q0 + 2 * g + 2)
            m[f"wq{g}"] = wlayout((Wq[hs].reshape(P, D) * NORM).astype(np.float32))
            m[f"wk{g}"] = wlayout(Wk[hs].reshape(P, D).astype(np.float32))
            m[f"wv{g}"] = wlayout(Wv[hs].reshape(P, D).astype(np.float32))
            m[f"bq{g}"] = (bq[hs].reshape(P, 1) * NORM).astype(np.float32)
            m[f"bk{g}"] = bk[hs].reshape(P, 1).astype(np.float32)
            m[f"bv{g}"] = bv[hs].reshape(P, 1).astype(np.float32)
        in_maps.append(m)
    return in_maps


def _gather(results):
    out = np.empty((B, S, D), dtype=np.float32)
    for c in range(NCORES):
        b = c // NB
        oc = results[c]["outT"]                             # [NG, P, S]
        for g in range(NG):
            f0 = (c % NB) * (HL * E) + g * P
            out[b, :, f0:f0 + P] = oc[g].T
    return out


def get_compiled():
    global _compiled
    if _compiled is None:
        _compiled = _build()
    return _compiled


def run(in_maps):
    nc = get_compiled()
    return bass2jax.run_bass_via_pjrt(nc, in_maps, n_cores=NCORES)


def kernel(x, Wq, Wk, Wv, bq, bk, bv):
    in_maps = _prep_inputs(
        np.asarray(x, np.float32), np.asarray(Wq, np.float32),
        np.asarray(Wk, np.float32), np.asarray(Wv, np.float32),
        np.asarray(bq, np.float32), np.asarray(bk, np.float32),
        np.asarray(bv, np.float32),
    )
    return _gather(run(in_maps))


# revision 38
# speedup vs baseline: 1.0354x; 1.0354x over previous
"""Multi-head attention (no softmax) on 8 TRN2 NeuronCores.

Problem: x[2,2048,1024], per-head Wq/Wk/Wv[16,64,1024] + biases.
    q = einsum('bsd,hed->bhse', x, Wq) + bq   (same for k, v)
    out = ((q @ k^T) * E^-0.5) @ v, heads concatenated on feature dim.

Key algebraic fact: there is NO softmax, so
    (q k^T * norm) v = q @ (norm * (k^T v))
which collapses the O(S^2) attention into a 64x64 (per head) matmul.

Sharding: 2D tensor-parallel over (batch, head-quad): core c owns batch
c//4 and heads 4*(c%4) .. 4*(c%4)+3.  Each core reads only its batch's
half of x (8.4MB), and processes its 4 heads as two packed head-pairs
(feature groups g=0,1 of 128).

Per core (all matmuls fp32r = fp32 storage, ~fp22 multiply, fp32 psum):
  phase 1: project QT/KT/VT[g] = W[g] @ x_b^T in [feat(128), seq(2048)]
           layout (N=512 moving dim -> full PE speed).  norm is folded
           into Wq/bq on the host; biases are per-partition adds fused
           into the PSUM->SBUF copies.  j-outer / g-inner so each DMA'd
           x chunk feeds both groups (the x stream is HBM-bound).
  phase 2: trailing one seq-chunk behind phase 1: PE-transpose K/V tiles
           to [seq, feat] and accumulate M_g = K^T V [128,128] in PSUM
           over 16 seq-chunks (M matmul software-pipelined one step
           behind its transposes); copy the two diagonal 64x64 head
           blocks into a zeroed SBUF tile (cross-head blocks of M are
           garbage and must be dropped).
  phase 3: outT[g][:, s-chunk] = M_g(blockdiag).T @ QT[g][:, s-chunk].
Host gathers: out[c//4, s, (c%4)*256+g*128 +:128] = outT_c[g][:, s].T

Scheduling notes (why the DMA/program order looks the way it does):
  - one HWDGE dispatch is ~650ns regardless of size; the Sync queue
    carries only wq0-chunks interleaved with x(:,0), then wk0/wv0, then
    the rest of x.  Ident, biases and g=1 weights ride the ACT HWDGE
    queue (needed later; keeping them off the x stream matters).
  - dummy bf16 matmuls at kernel start hold the PE's HAM clock gate
    open (cold PE runs at 1.2GHz for its first ~3.4us) while the first
    x tiles land.
"""

import numpy as np

import concourse.bacc as bacc
import concourse.tile as tile
import concourse.mybir as mybir
from concourse import bass2jax

B, S, D, H = 2, 2048, 1024, 16
E = 64          # head dim
NCORES = 8
NB = NCORES // B            # cores per batch (4)
HL = H // NB                # heads per core (4)
NG = 2                      # feature groups per core (head pairs)
EP = HL * E // NG           # packed feature dim per group (128)
P = 128                     # partitions
DC = D // P                 # d chunks (8)
SC = 512                    # seq chunk for N=512 matmuls
NSC = S // SC               # 4 seq chunks
TC = S // P                 # 16 transpose chunks per group
NORM = float(E) ** -0.5

F32 = mybir.dt.float32
F32R = mybir.dt.float32r

_compiled = None


def _build():
    nc = bacc.Bacc("TRN2", target_bir_lowering=False, debug=False)

    x_d = nc.dram_tensor("x", [DC, NSC, P, SC], F32R, kind="ExternalInput").ap()
    w_d = {}
    for g in range(NG):
        for wn in ("wq", "wk", "wv"):
            w_d[wn, g] = nc.dram_tensor(
                f"{wn}{g}", [P, DC, P], F32R, kind="ExternalInput").ap()
    b_d = {}
    for g in range(NG):
        for bn in ("bq", "bk", "bv"):
            b_d[bn, g] = nc.dram_tensor(
                f"{bn}{g}", [P, 1], F32, kind="ExternalInput").ap()
    id_d = nc.dram_tensor("ident", [P, P], F32R, kind="ExternalInput").ap()
    out_d = nc.dram_tensor("outT", [NG, P, S], F32, kind="ExternalOutput").ap()

    with tile.TileContext(nc) as tc:
        with (
            tc.tile_pool(name="consts", bufs=1) as consts,
            tc.tile_pool(name="xs", bufs=32) as xs_pool,
            tc.tile_pool(name="qkv", bufs=1) as qkv_pool,
            tc.tile_pool(name="kv", bufs=8) as kv_pool,
            tc.tile_pool(name="mt", bufs=1) as mt_pool,
            tc.tile_pool(name="ot", bufs=8) as ot_pool,
            tc.tile_pool(name="pproj", bufs=6, space="PSUM") as pproj,
            tc.tile_pool(name="pm", bufs=2, space="PSUM") as pm,
        ):
            # ---- input DMAs (order matters; see scheduling notes above)
            xs = {}

            def load_x(i, j):
                t = xs_pool.tile([P, SC], F32R, tag="xs", name=f"x_{i}_{j}", bufs=16)
                nc.sync.dma_start(t[:], x_d[i, j])
                xs[i, j] = t[:]

            def load_x_pair(i):
                # j=2,3 ride one 512KB DMA: the HWDGE queue costs ~650ns
                # dispatch per DMA regardless of size, so the back half of
                # the x stream finishes ~5us earlier as 8 paired transfers
                t = xs_pool.tile([P, 2, SC], F32R, tag="xs2", name=f"x2_{i}", bufs=8)
                nc.sync.dma_start(t[:], x_d[i, 2:4].rearrange("a p s -> p a s"))
                xs[i, 2] = t[:, 0, :]
                xs[i, 3] = t[:, 1, :]

            w_tiles, b_tiles = {}, {}

            def load_w(wn, g):
                wt = consts.tile([P, DC, P], F32R, tag=f"{wn}{g}", name=f"{wn}{g}_t")
                nc.sync.dma_start(wt[:], w_d[wn, g][:])
                w_tiles[wn, g] = wt

            # interleave the first weight chunks with the first x tiles so
            # the first accumulation group can start after ~0.5MB of DMA
            wq0 = consts.tile([P, DC, P], F32R, tag="wq0", name="wq0_t")
            w_tiles["wq", 0] = wq0
            for i in range(DC):
                nc.sync.dma_start(wq0[:, i, :], w_d["wq", 0][:, i, :])
                load_x(i, 0)
            load_w("wk", 0)
            load_w("wv", 0)
            # Biases, ident and g=1 weights ride the ACT HWDGE queue so the
            # Sync queue stays a pure x stream (it is HBM-bound; every DMA
            # inserted into it delays the x tiles phase 1 is waiting for).
            # Order: ident + g=1 weights are needed within ~10-16us (the
            # j-interleaved schedule uses both groups' weights early).
            ident = consts.tile([P, P], F32R, tag="ident")
            nc.scalar.dma_start(ident[:], id_d[:])
            for wn in ("wq", "wk", "wv"):
                wt = consts.tile([P, DC, P], F32R, tag=f"{wn}1", name=f"{wn}1_t")
                nc.scalar.dma_start(wt[:], w_d[wn, 1][:])
                w_tiles[wn, 1] = wt
            for bn in ("bq", "bk", "bv"):
                for g in range(NG):
                    bt = consts.tile([P, 1], F32, tag=f"{bn}{g}", name=f"{bn}{g}_t")
                    nc.scalar.dma_start(bt[:], b_d[bn, g][:])
                    b_tiles[bn, g] = bt
            for i in range(DC):
                load_x(i, 1)
            for i in range(DC):
                load_x_pair(i)

            # ---- PE clock warmup: the HAM gate holds the PE at 1.2GHz for
            # its first ~3.4us of sustained activity, and the PE is idle
            # until the first x tiles land anyway.  Burn that window with
            # dummy matmuls on a zeroed scratch tile so the real matmuls
            # start at 2.4GHz.  Results go to a psum tile nobody reads.
            warm_in = consts.tile([P, 256], mybir.dt.bfloat16, tag="warm")
            nc.gpsimd.memset(warm_in[:], 0.0)
            warm_ps = pproj.tile([P, 256], F32, tag="proj", name="warm_ps")
            for _ in range(10):
                nc.tensor.matmul(warm_ps[:], warm_in[:, 0:P], warm_in[:],
                                 start=True, stop=True)

            # ---- phases 1+2, transposes trailing one seq chunk
            big = {}
            for g in range(NG):
                for tn in ("q", "k", "v"):
                    big[tn, g] = qkv_pool.tile([P, S], F32R, tag=f"{tn}t{g}",
                                               name=f"{tn}t{g}")
            m_psum = {}
            pending = {g: None for g in range(NG)}  # per-g pipelined M matmul

            def transpose_chunk(g, j):
                for tt in range(SC // P):
                    t = j * (SC // P) + tt
                    sl = slice(t * P, (t + 1) * P)
                    ktp = pproj.tile([P, P], F32R, tag="proj",
                                     name=f"ktp_{g}_{t}")
                    nc.tensor.transpose(ktp[:], big["k", g][:, sl], ident[:])
                    k_sb = kv_pool.tile([P, P], F32R, tag="k_sb",
                                        name=f"k_sb_{g}_{t}")
                    nc.scalar.copy(k_sb[:], ktp[:])
                    vtp = pproj.tile([P, P], F32R, tag="proj",
                                     name=f"vtp_{g}_{t}")
                    nc.tensor.transpose(vtp[:], big["v", g][:, sl], ident[:])
                    v_sb = kv_pool.tile([P, P], F32R, tag="v_sb",
                                        name=f"v_sb_{g}_{t}")
                    nc.vector.tensor_copy(v_sb[:], vtp[:])
                    if pending[g] is not None:
                        p = pending[g]
                        nc.tensor.matmul(
                            m_psum[g][:], p[0][:], p[1][:],
                            start=(p[2] == 0), stop=(p[2] == TC - 1),
                            skip_group_check=True)
                    pending[g] = (k_sb, v_sb, t)

            def proj_group(tn, wn, bn, g, j, filler=0):
                ps = pproj.tile([P, SC], F32, tag="proj",
                                name=f"ps_{tn}{g}_{j}")
                for i in range(DC):
                    nc.tensor.matmul(
                        ps[:], w_tiles[wn, g][:, i, :], xs[i, j],
                        start=(i == 0), stop=(i == DC - 1),
                        skip_group_check=bool(filler),
                    )
                    # dummy matmuls between the accumulation steps: during
                    # the first chunk the x tiles land one DMA (~650ns)
                    # apart, slower than the PE consumes them, so the PE is
                    # provably idle here; the filler keeps the HAM clock
                    # gate open and soaks up the wait.
                    if filler and i < DC - 1:
                        for _ in range(filler):
                            nc.tensor.matmul(warm_ps[:], warm_in[:, 0:P],
                                             warm_in[:], start=True, stop=True,
                                             skip_group_check=True)
                sl = big[tn, g][:, j * SC:(j + 1) * SC]
                if tn == "v":
                    nc.scalar.activation(
                        sl, ps[:], mybir.ActivationFunctionType.Identity,
                        bias=b_tiles[bn, g][:])
                else:
                    nc.vector.tensor_scalar_add(sl, ps[:], b_tiles[bn, g][:])

            # j-outer, tensor-outer, g-inner: each x chunk feeds BOTH groups'
            # projections (48 matmuls per 2MB of DMA), so the HBM-bound x
            # stream never starves the PE after the first chunk, and each
            # weight tensor is first needed in the order the two DMA queues
            # deliver them (wq0 | wq1, wk1, wv1 (ACT) | wk0, wv0 (sync)).
            # Transposes trail one chunk behind.
            for g in range(NG):
                m_psum[g] = pm.tile([P, P], F32, tag="m", name=f"mps_{g}")
            for j in range(NSC):
                for g in range(NG):
                    for tn, wn, bn in (("q", "wq", "bq"), ("k", "wk", "bk"),
                                       ("v", "wv", "bv")):
                        proj_group(tn, wn, bn, g, j)
                for g in range(NG):
                    if j > 0:
                        transpose_chunk(g, j - 1)
            for g in range(NG):
                transpose_chunk(g, NSC - 1)
                p = pending[g]
                nc.tensor.matmul(m_psum[g][:], p[0][:], p[1][:],
                                 start=(p[2] == 0), stop=True,
                                 skip_group_check=True)
                mt = mt_pool.tile([P, P], F32R, tag=f"mt{g}", name=f"mt_{g}")
                # zero-fill without InstMemset (walrus rejects f32r memset)
                nc.vector.tensor_scalar_mul(mt[:], ident[:], 0.0)
                nc.vector.tensor_copy(mt[0:E, 0:E], m_psum[g][0:E, 0:E])
                nc.vector.tensor_copy(mt[E:P, E:P], m_psum[g][E:P, E:P])

                # phase 3 for this group
                for j in range(NSC):
                    sl = slice(j * SC, (j + 1) * SC)
                    ps = pproj.tile([P, SC], F32, tag="proj", name=f"ops_{g}_{j}")
                    nc.tensor.matmul(ps[:], mt[:], big["q", g][:, sl],
                                     start=True, stop=True)
                    ot = ot_pool.tile([P, SC], F32, tag="ot", name=f"ot_{g}_{j}")
                    nc.vector.tensor_copy(ot[:], ps[:])
                    nc.sync.dma_start(out_d[g, :, sl], ot[:])

    nc.compile()
    return nc


def _prep_inputs(x, Wq, Wk, Wv, bq, bk, bv):
    """Host-side shard + layout prep. Returns per-core input maps."""
    x_tiles_b = []
    for b in range(B):
        xf = np.ascontiguousarray(x[b].T)                   # [D, S]
        x_tiles_b.append(np.ascontiguousarray(
            xf.reshape(DC, P, NSC, SC).transpose(0, 2, 1, 3)))

    def wlayout(w):                                         # [P, D] -> [P, DC, P]
        return np.ascontiguousarray(w.T.reshape(DC, P, P).transpose(1, 0, 2))

    in_maps = []
    for c in range(NCORES):
        b = c // NB
        q0 = HL * (c % NB)                                  # first head of core
        m = {"x": x_tiles_b[b], "ident": np.eye(P, dtype=np.float32)}
        for g in range(NG):
            hs = slice(q0 + 2 * g, q0 + 2 * g + 2)
            m[f"wq{g}"] = wlayout((Wq[hs].reshape(P, D) * NORM).astype(np.float32))
            m[f"wk{g}"] = wlayout(Wk[hs].reshape(P, D).astype(np.float32))
            m[f"wv{g}"] = wlayout(Wv[hs].reshape(P, D).astype(np.float32))
            m[f"bq{g}"] = (bq[hs].reshape(P, 1) * NORM).astype(np.float32)
            m[f"bk{g}"] = bk[hs].reshape(P, 1).astype(np.float32)
            m[f"bv{g}"] = bv[hs].reshape(P, 1).astype(np.float32)
        in_maps.append(m)
    return in_maps


def _gather(results):
    out = np.empty((B, S, D), dtype=np.float32)
    for c in range(NCORES):
        b = c // NB
        oc = results[c]["outT"]                             # [NG, P, S]
        for g in range(NG):
            f0 = (c % NB) * (HL * E) + g * P
            out[b, :, f0:f0 + P] = oc[g].T
    return out


def get_compiled():
    global _compiled
    if _compiled is None:
        _compiled = _build()
    return _compiled


def run(in_maps):
    nc = get_compiled()
    return bass2jax.run_bass_via_pjrt(nc, in_maps, n_cores=NCORES)


def kernel(x, Wq, Wk, Wv, bq, bk, bv):
    in_maps = _prep_inputs(
        np.asarray(x, np.float32), np.asarray(Wq, np.float32),
        np.asarray(Wk, np.float32), np.asarray(Wv, np.float32),
        np.asarray(bq, np.float32), np.asarray(bk, np.float32),
        np.asarray(bv, np.float32),
    )
    return _gather(run(in_maps))


# revision 39
# speedup vs baseline: 1.0552x; 1.0191x over previous
"""Multi-head attention (no softmax) on 8 TRN2 NeuronCores.

Problem: x[2,2048,1024], per-head Wq/Wk/Wv[16,64,1024] + biases.
    q = einsum('bsd,hed->bhse', x, Wq) + bq   (same for k, v)
    out = ((q @ k^T) * E^-0.5) @ v, heads concatenated on feature dim.

Key algebraic fact: there is NO softmax, so
    (q k^T * norm) v = q @ (norm * (k^T v))
which collapses the O(S^2) attention into a 64x64 (per head) matmul.

Sharding: 2D tensor-parallel over (batch, head-quad): core c owns batch
c//4 and heads 4*(c%4) .. 4*(c%4)+3.  Each core reads only its batch's
half of x (8.4MB), and processes its 4 heads as two packed head-pairs
(feature groups g=0,1 of 128).

Per core (all matmuls fp32r = fp32 storage, ~fp22 multiply, fp32 psum):
  phase 1: project QT/KT/VT[g] = W[g] @ x_b^T in [feat(128), seq(2048)]
           layout (N=512 moving dim -> full PE speed).  norm is folded
           into Wq/bq on the host; biases are per-partition adds fused
           into the PSUM->SBUF copies.  j-outer / g-inner so each DMA'd
           x chunk feeds both groups (the x stream is HBM-bound).
  phase 2: trailing one seq-chunk behind phase 1: PE-transpose K/V tiles
           to [seq, feat] and accumulate M_g = K^T V [128,128] in PSUM
           over 16 seq-chunks (M matmul software-pipelined one step
           behind its transposes); copy the two diagonal 64x64 head
           blocks into a zeroed SBUF tile (cross-head blocks of M are
           garbage and must be dropped).
  phase 3: outT[g][:, s-chunk] = M_g(blockdiag).T @ QT[g][:, s-chunk].
Host gathers: out[c//4, s, (c%4)*256+g*128 +:128] = outT_c[g][:, s].T

Scheduling notes (why the DMA/program order looks the way it does):
  - one HWDGE dispatch is ~650ns regardless of size; the Sync queue
    carries only wq0-chunks interleaved with x(:,0), then wk0/wv0, then
    the rest of x.  Ident, biases and g=1 weights ride the ACT HWDGE
    queue (needed later; keeping them off the x stream matters).
  - dummy bf16 matmuls at kernel start hold the PE's HAM clock gate
    open (cold PE runs at 1.2GHz for its first ~3.4us) while the first
    x tiles land.
"""

import numpy as np

import concourse.bacc as bacc
import concourse.tile as tile
import concourse.mybir as mybir
from concourse import bass2jax

B, S, D, H = 2, 2048, 1024, 16
E = 64          # head dim
NCORES = 8
NB = NCORES // B            # cores per batch (4)
HL = H // NB                # heads per core (4)
NG = 2                      # feature groups per core (head pairs)
EP = HL * E // NG           # packed feature dim per group (128)
P = 128                     # partitions
DC = D // P                 # d chunks (8)
SC = 512                    # seq chunk for N=512 matmuls
NSC = S // SC               # 4 seq chunks
TC = S // P                 # 16 transpose chunks per group
NORM = float(E) ** -0.5

F32 = mybir.dt.float32
F32R = mybir.dt.float32r

_compiled = None


def _build():
    nc = bacc.Bacc("TRN2", target_bir_lowering=False, debug=False)

    x_d = nc.dram_tensor("x", [DC, NSC, P, SC], F32R, kind="ExternalInput").ap()
    w_d = {}
    for g in range(NG):
        for wn in ("wq", "wk", "wv"):
            w_d[wn, g] = nc.dram_tensor(
                f"{wn}{g}", [P, DC, P], F32R, kind="ExternalInput").ap()
    b_d = {}
    for g in range(NG):
        for bn in ("bq", "bk", "bv"):
            b_d[bn, g] = nc.dram_tensor(
                f"{bn}{g}", [P, 1], F32, kind="ExternalInput").ap()
    id_d = nc.dram_tensor("ident", [P, P], F32R, kind="ExternalInput").ap()
    out_d = nc.dram_tensor("outT", [NG, P, S], F32, kind="ExternalOutput").ap()

    with tile.TileContext(nc) as tc:
        with (
            tc.tile_pool(name="consts", bufs=1) as consts,
            tc.tile_pool(name="xs", bufs=32) as xs_pool,
            tc.tile_pool(name="qkv", bufs=1) as qkv_pool,
            tc.tile_pool(name="kv", bufs=8) as kv_pool,
            tc.tile_pool(name="mt", bufs=1) as mt_pool,
            tc.tile_pool(name="ot", bufs=8) as ot_pool,
            tc.tile_pool(name="pproj", bufs=6, space="PSUM") as pproj,
            tc.tile_pool(name="pm", bufs=2, space="PSUM") as pm,
        ):
            # ---- input DMAs (order matters; see scheduling notes above)
            xs = {}

            def load_x(i, j):
                t = xs_pool.tile([P, SC], F32R, tag="xs", name=f"x_{i}_{j}", bufs=16)
                nc.sync.dma_start(t[:], x_d[i, j])
                xs[i, j] = t[:]

            def load_x_pair(i):
                # j=2,3 ride one 512KB DMA: the HWDGE queue costs ~650ns
                # dispatch per DMA regardless of size, so the back half of
                # the x stream finishes ~5us earlier as 8 paired transfers
                t = xs_pool.tile([P, 2, SC], F32R, tag="xs2", name=f"x2_{i}", bufs=8)
                nc.sync.dma_start(t[:], x_d[i, 2:4].rearrange("a p s -> p a s"))
                xs[i, 2] = t[:, 0, :]
                xs[i, 3] = t[:, 1, :]

            w_tiles, b_tiles = {}, {}

            def load_w(wn, g):
                wt = consts.tile([P, DC, P], F32R, tag=f"{wn}{g}", name=f"{wn}{g}_t")
                nc.sync.dma_start(wt[:], w_d[wn, g][:])
                w_tiles[wn, g] = wt

            # interleave the first weight chunks with the first x tiles so
            # the first accumulation group can start after ~0.5MB of DMA
            wq0 = consts.tile([P, DC, P], F32R, tag="wq0", name="wq0_t")
            w_tiles["wq", 0] = wq0
            for i in range(DC):
                nc.sync.dma_start(wq0[:, i, :], w_d["wq", 0][:, i, :])
                load_x(i, 0)
            load_w("wk", 0)
            load_w("wv", 0)
            # Biases, ident and g=1 weights ride the ACT HWDGE queue so the
            # Sync queue stays a pure x stream (it is HBM-bound; every DMA
            # inserted into it delays the x tiles phase 1 is waiting for).
            # Order: ident + g=1 weights are needed within ~10-16us (the
            # j-interleaved schedule uses both groups' weights early).
            ident = consts.tile([P, P], F32R, tag="ident")
            nc.scalar.dma_start(ident[:], id_d[:])
            for wn in ("wq", "wk", "wv"):
                wt = consts.tile([P, DC, P], F32R, tag=f"{wn}1", name=f"{wn}1_t")
                nc.scalar.dma_start(wt[:], w_d[wn, 1][:])
                w_tiles[wn, 1] = wt
            for bn in ("bq", "bk", "bv"):
                for g in range(NG):
                    bt = consts.tile([P, 1], F32, tag=f"{bn}{g}", name=f"{bn}{g}_t")
                    nc.scalar.dma_start(bt[:], b_d[bn, g][:])
                    b_tiles[bn, g] = bt
            for i in range(DC):
                load_x(i, 1)
            for i in range(DC):
                load_x_pair(i)

            # ---- PE clock warmup: the HAM gate holds the PE at 1.2GHz for
            # its first ~3.4us of sustained activity, and the PE is idle
            # until the first x tiles land anyway.  Burn that window with
            # dummy matmuls on a zeroed scratch tile so the real matmuls
            # start at 2.4GHz.  Results go to a psum tile nobody reads.
            warm_in = consts.tile([P, 256], mybir.dt.bfloat16, tag="warm")
            nc.gpsimd.memset(warm_in[:], 0.0)
            warm_ps = pproj.tile([P, 256], F32, tag="proj", name="warm_ps")
            for _ in range(10):
                nc.tensor.matmul(warm_ps[:], warm_in[:, 0:P], warm_in[:],
                                 start=True, stop=True)

            # ---- phases 1+2, transposes trailing one seq chunk
            big = {}
            for g in range(NG):
                for tn in ("q", "k", "v"):
                    big[tn, g] = qkv_pool.tile([P, S], F32R, tag=f"{tn}t{g}",
                                               name=f"{tn}t{g}")
            m_psum = {}
            pending = {g: None for g in range(NG)}  # per-g pipelined M matmul

            def transpose_chunk(g, j):
                for tt in range(SC // P):
                    t = j * (SC // P) + tt
                    sl = slice(t * P, (t + 1) * P)
                    ktp = pproj.tile([P, P], F32R, tag="proj",
                                     name=f"ktp_{g}_{t}")
                    nc.tensor.transpose(ktp[:], big["k", g][:, sl], ident[:])
                    k_sb = kv_pool.tile([P, P], F32R, tag="k_sb",
                                        name=f"k_sb_{g}_{t}")
                    nc.scalar.copy(k_sb[:], ktp[:])
                    vtp = pproj.tile([P, P], F32R, tag="proj",
                                     name=f"vtp_{g}_{t}")
                    nc.tensor.transpose(vtp[:], big["v", g][:, sl], ident[:])
                    v_sb = kv_pool.tile([P, P], F32R, tag="v_sb",
                                        name=f"v_sb_{g}_{t}")
                    nc.vector.tensor_copy(v_sb[:], vtp[:])
                    if pending[g] is not None:
                        p = pending[g]
                        nc.tensor.matmul(
                            m_psum[g][:], p[0][:], p[1][:],
                            start=(p[2] == 0), stop=(p[2] == TC - 1),
                            skip_group_check=True)
                    pending[g] = (k_sb, v_sb, t)

            def proj_group(tn, wn, bn, g, j, filler=0):
                ps = pproj.tile([P, SC], F32, tag="proj",
                                name=f"ps_{tn}{g}_{j}")
                for i in range(DC):
                    nc.tensor.matmul(
                        ps[:], w_tiles[wn, g][:, i, :], xs[i, j],
                        start=(i == 0), stop=(i == DC - 1),
                        skip_group_check=bool(filler),
                    )
                    # dummy matmuls between the accumulation steps: during
                    # the first chunk the x tiles land one DMA (~650ns)
                    # apart, slower than the PE consumes them, so the PE is
                    # provably idle here; the filler keeps the HAM clock
                    # gate open and soaks up the wait.
                    if filler and i < DC - 1:
                        for _ in range(filler):
                            nc.tensor.matmul(warm_ps[:], warm_in[:, 0:P],
                                             warm_in[:], start=True, stop=True,
                                             skip_group_check=True)
                sl = big[tn, g][:, j * SC:(j + 1) * SC]
                if tn == "v":
                    nc.scalar.activation(
                        sl, ps[:], mybir.ActivationFunctionType.Identity,
                        bias=b_tiles[bn, g][:])
                else:
                    nc.vector.tensor_scalar_add(sl, ps[:], b_tiles[bn, g][:])

            # j-outer, tensor-outer, g-inner: each x chunk feeds BOTH groups'
            # projections (48 matmuls per 2MB of DMA), so the HBM-bound x
            # stream never starves the PE after the first chunk, and each
            # weight tensor is first needed in the order the two DMA queues
            # deliver them (wq0 | wq1, wk1, wv1 (ACT) | wk0, wv0 (sync)).
            # Transposes trail one chunk behind.
            for g in range(NG):
                m_psum[g] = pm.tile([P, P], F32, tag="m", name=f"mps_{g}")
            for j in range(NSC):
                for g in range(NG):
                    for tn, wn, bn in (("q", "wq", "bq"), ("k", "wk", "bk"),
                                       ("v", "wv", "bv")):
                        proj_group(tn, wn, bn, g, j)
                for g in range(NG):
                    if j > 0:
                        transpose_chunk(g, j - 1)
            for g in range(NG):
                transpose_chunk(g, NSC - 1)
                p = pending[g]
                nc.tensor.matmul(m_psum[g][:], p[0][:], p[1][:],
                                 start=(p[2] == 0), stop=True,
                                 skip_group_check=True)
                mt = mt_pool.tile([P, P], F32R, tag=f"mt{g}", name=f"mt_{g}")
                # zero-fill without InstMemset (walrus rejects f32r memset)
                nc.vector.tensor_scalar_mul(mt[:], ident[:], 0.0)
                nc.vector.tensor_copy(mt[0:E, 0:E], m_psum[g][0:E, 0:E])
                nc.vector.tensor_copy(mt[E:P, E:P], m_psum[g][E:P, E:P])

                # phase 3 for this group
                for j in range(NSC):
                    sl = slice(j * SC, (j + 1) * SC)
                    ps = pproj.tile([P, SC], F32, tag="proj", name=f"ops_{g}_{j}")
                    nc.tensor.matmul(ps[:], mt[:], big["q", g][:, sl],
                                     start=True, stop=True)
                    ot = ot_pool.tile([P, SC], F32, tag="ot", name=f"ot_{g}_{j}")
                    if j % 2 == 0:
                        nc.vector.tensor_copy(ot[:], ps[:])
                    else:
                        nc.scalar.copy(ot[:], ps[:])
                    nc.sync.dma_start(out_d[g, :, sl], ot[:])

    nc.compile()
    return nc


def _prep_inputs(x, Wq, Wk, Wv, bq, bk, bv):
    """Host-side shard + layout prep. Returns per-core input maps."""
    x_tiles_b = []
    for b in range(B):
        xf = np.ascontiguousarray(x[b].T)                   # [D, S]
        x_tiles_b.append(np.ascontiguousarray(
            xf.reshape(DC, P, NSC, SC).transpose(0, 2, 1, 3)))

    def wlayout(w):                                         # [P, D] -> [P, DC, P]
        return np.ascontiguousarray(w.T.reshape(DC, P, P).transpose(1, 0, 2))

    in_maps = []
    for c in range(NCORES):
        b = c // NB
        q0 = HL * (c % NB)                                  # first head of core
        m = {"x": x_tiles_b[b], "ident": np.eye(P, dtype=np.float32)}
        for g in range(NG):
            hs = slice(q0 + 2 * g, q0 + 2 * g + 2)
            m[f"wq{g}"] = wlayout((Wq[hs].reshape(P, D) * NORM).astype(np.float32))
            m[f"wk{g}"] = wlayout(Wk[hs].reshape(P, D).astype(np.float32))
            m[f"wv{g}"] = wlayout(Wv[hs].reshape(P, D).astype(np.float32))
            m[f"bq{g}"] = (bq[hs].reshape(P, 1) * NORM).astype(np.float32)
            m[f"bk{g}"] = bk[hs].reshape(P, 1).astype(np.float32)
            m[f"bv{g}"] = bv[hs].reshape(P, 1).astype(np.float32)
        in_maps.append(m)
    return in_maps


def _gather(results):
    out = np.empty((B, S, D), dtype=np.float32)
    for c in range(NCORES):
        b = c // NB
        oc = results[c]["outT"]                             # [NG, P, S]
        for g in range(NG):
            f0 = (c % NB) * (HL * E) + g * P
            out[b, :, f0:f0 + P] = oc[g].T
    return out


def get_compiled():
    global _compiled
    if _compiled is None:
        _compiled = _build()
    return _compiled


def run(in_maps):
    nc = get_compiled()
    return bass2jax.run_bass_via_pjrt(nc, in_maps, n_cores=NCORES)


def kernel(x, Wq, Wk, Wv, bq, bk, bv):
    in_maps = _prep_inputs(
        np.asarray(x, np.float32), np.asarray(Wq, np.float32),
        np.asarray(Wk, np.float32), np.asarray(Wv, np.float32),
        np.asarray(bq, np.float32), np.asarray(bk, np.float32),
        np.asarray(bv, np.float32),
    )
    return _gather(run(in_maps))


# revision 40
# speedup vs baseline: 1.0752x; 1.0189x over previous
"""Multi-head attention (no softmax) on 8 TRN2 NeuronCores.

Problem: x[2,2048,1024], per-head Wq/Wk/Wv[16,64,1024] + biases.
    q = einsum('bsd,hed->bhse', x, Wq) + bq   (same for k, v)
    out = ((q @ k^T) * E^-0.5) @ v, heads concatenated on feature dim.

Key algebraic fact: there is NO softmax, so
    (q k^T * norm) v = q @ (norm * (k^T v))
which collapses the O(S^2) attention into a 64x64 (per head) matmul.

Sharding: 2D tensor-parallel over (batch, head-quad): core c owns batch
c//4 and heads 4*(c%4) .. 4*(c%4)+3.  Each core reads only its batch's
half of x (8.4MB), and processes its 4 heads as two packed head-pairs
(feature groups g=0,1 of 128).

Per core (all matmuls fp32r = fp32 storage, ~fp22 multiply, fp32 psum):
  phase 1: project QT/KT/VT[g] = W[g] @ x_b^T in [feat(128), seq(2048)]
           layout (N=512 moving dim -> full PE speed).  norm is folded
           into Wq/bq on the host; biases are per-partition adds fused
           into the PSUM->SBUF copies.  j-outer / g-inner so each DMA'd
           x chunk feeds both groups (the x stream is HBM-bound).
  phase 2: trailing one seq-chunk behind phase 1: PE-transpose K/V tiles
           to [seq, feat] and accumulate M_g = K^T V [128,128] in PSUM
           over 16 seq-chunks (M matmul software-pipelined one step
           behind its transposes); copy the two diagonal 64x64 head
           blocks into a zeroed SBUF tile (cross-head blocks of M are
           garbage and must be dropped).
  phase 3: outT[g][:, s-chunk] = M_g(blockdiag).T @ QT[g][:, s-chunk].
Host gathers: out[c//4, s, (c%4)*256+g*128 +:128] = outT_c[g][:, s].T

Scheduling notes (why the DMA/program order looks the way it does):
  - one HWDGE dispatch is ~650ns regardless of size; the Sync queue
    carries only wq0-chunks interleaved with x(:,0), then wk0/wv0, then
    the rest of x.  Ident, biases and g=1 weights ride the ACT HWDGE
    queue (needed later; keeping them off the x stream matters).
  - dummy bf16 matmuls at kernel start hold the PE's HAM clock gate
    open (cold PE runs at 1.2GHz for its first ~3.4us) while the first
    x tiles land.
"""

import numpy as np

import concourse.bacc as bacc
import concourse.tile as tile
import concourse.mybir as mybir
from concourse import bass2jax

B, S, D, H = 2, 2048, 1024, 16
E = 64          # head dim
NCORES = 8
NB = NCORES // B            # cores per batch (4)
HL = H // NB                # heads per core (4)
NG = 2                      # feature groups per core (head pairs)
EP = HL * E // NG           # packed feature dim per group (128)
P = 128                     # partitions
DC = D // P                 # d chunks (8)
SC = 512                    # seq chunk for N=512 matmuls
NSC = S // SC               # 4 seq chunks
TC = S // P                 # 16 transpose chunks per group
NORM = float(E) ** -0.5

F32 = mybir.dt.float32
F32R = mybir.dt.float32r

_compiled = None


def _build():
    nc = bacc.Bacc("TRN2", target_bir_lowering=False, debug=False)

    x_d = nc.dram_tensor("x", [DC, NSC, P, SC], F32R, kind="ExternalInput").ap()
    w_d = {}
    for g in range(NG):
        for wn in ("wq", "wk", "wv"):
            w_d[wn, g] = nc.dram_tensor(
                f"{wn}{g}", [P, DC, P], F32R, kind="ExternalInput").ap()
    b_d = {}
    for g in range(NG):
        for bn in ("bq", "bk", "bv"):
            b_d[bn, g] = nc.dram_tensor(
                f"{bn}{g}", [P, 1], F32, kind="ExternalInput").ap()
    id_d = nc.dram_tensor("ident", [P, P], F32R, kind="ExternalInput").ap()
    out_d = nc.dram_tensor("outT", [NG, P, S], F32, kind="ExternalOutput").ap()

    with tile.TileContext(nc) as tc:
        with (
            tc.tile_pool(name="consts", bufs=1) as consts,
            tc.tile_pool(name="xs", bufs=32) as xs_pool,
            tc.tile_pool(name="qkv", bufs=1) as qkv_pool,
            tc.tile_pool(name="kv", bufs=8) as kv_pool,
            tc.tile_pool(name="mt", bufs=1) as mt_pool,
            tc.tile_pool(name="ot", bufs=8) as ot_pool,
            tc.tile_pool(name="pproj", bufs=6, space="PSUM") as pproj,
            tc.tile_pool(name="pm", bufs=2, space="PSUM") as pm,
        ):
            # ---- input DMAs (order matters; see scheduling notes above)
            xs = {}

            def load_x(i, j):
                t = xs_pool.tile([P, SC], F32R, tag="xs", name=f"x_{i}_{j}", bufs=16)
                nc.sync.dma_start(t[:], x_d[i, j])
                xs[i, j] = t[:]

            def load_x_pair(i):
                # j=2,3 ride one 512KB DMA: the HWDGE queue costs ~650ns
                # dispatch per DMA regardless of size, so the back half of
                # the x stream finishes ~5us earlier as 8 paired transfers
                t = xs_pool.tile([P, 2, SC], F32R, tag="xs2", name=f"x2_{i}", bufs=8)
                nc.sync.dma_start(t[:], x_d[i, 2:4].rearrange("a p s -> p a s"))
                xs[i, 2] = t[:, 0, :]
                xs[i, 3] = t[:, 1, :]

            w_tiles, b_tiles = {}, {}

            def load_w(wn, g):
                wt = consts.tile([P, DC, P], F32R, tag=f"{wn}{g}", name=f"{wn}{g}_t")
                nc.sync.dma_start(wt[:], w_d[wn, g][:])
                w_tiles[wn, g] = wt

            # interleave the first weight chunks with the first x tiles so
            # the first accumulation group can start after ~0.5MB of DMA
            wq0 = consts.tile([P, DC, P], F32R, tag="wq0", name="wq0_t")
            w_tiles["wq", 0] = wq0
            nc.sync.dma_start(wq0[:, 0:4, :], w_d["wq", 0][:, 0:4, :])
            load_x(0, 0)
            nc.sync.dma_start(wq0[:, 4:8, :], w_d["wq", 0][:, 4:8, :])
            for i in range(1, DC):
                load_x(i, 0)
            load_w("wk", 0)
            load_w("wv", 0)
            # Biases, ident and g=1 weights ride the ACT HWDGE queue so the
            # Sync queue stays a pure x stream (it is HBM-bound; every DMA
            # inserted into it delays the x tiles phase 1 is waiting for).
            # Order: ident + g=1 weights are needed within ~10-16us (the
            # j-interleaved schedule uses both groups' weights early).
            ident = consts.tile([P, P], F32R, tag="ident")
            nc.scalar.dma_start(ident[:], id_d[:])
            for wn in ("wq", "wk", "wv"):
                wt = consts.tile([P, DC, P], F32R, tag=f"{wn}1", name=f"{wn}1_t")
                nc.scalar.dma_start(wt[:], w_d[wn, 1][:])
                w_tiles[wn, 1] = wt
            for bn in ("bq", "bk", "bv"):
                for g in range(NG):
                    bt = consts.tile([P, 1], F32, tag=f"{bn}{g}", name=f"{bn}{g}_t")
                    nc.scalar.dma_start(bt[:], b_d[bn, g][:])
                    b_tiles[bn, g] = bt
            for i in range(DC):
                load_x(i, 1)
            for i in range(DC):
                load_x_pair(i)

            # ---- PE clock warmup: the HAM gate holds the PE at 1.2GHz for
            # its first ~3.4us of sustained activity, and the PE is idle
            # until the first x tiles land anyway.  Burn that window with
            # dummy matmuls on a zeroed scratch tile so the real matmuls
            # start at 2.4GHz.  Results go to a psum tile nobody reads.
            warm_in = consts.tile([P, 256], mybir.dt.bfloat16, tag="warm")
            nc.gpsimd.memset(warm_in[:], 0.0)
            warm_ps = pproj.tile([P, 256], F32, tag="proj", name="warm_ps")
            for _ in range(10):
                nc.tensor.matmul(warm_ps[:], warm_in[:, 0:P], warm_in[:],
                                 start=True, stop=True)

            # ---- phases 1+2, transposes trailing one seq chunk
            big = {}
            for g in range(NG):
                for tn in ("q", "k", "v"):
                    big[tn, g] = qkv_pool.tile([P, S], F32R, tag=f"{tn}t{g}",
                                               name=f"{tn}t{g}")
            m_psum = {}
            pending = {g: None for g in range(NG)}  # per-g pipelined M matmul

            def transpose_chunk(g, j):
                for tt in range(SC // P):
                    t = j * (SC // P) + tt
                    sl = slice(t * P, (t + 1) * P)
                    ktp = pproj.tile([P, P], F32R, tag="proj",
                                     name=f"ktp_{g}_{t}")
                    nc.tensor.transpose(ktp[:], big["k", g][:, sl], ident[:])
                    k_sb = kv_pool.tile([P, P], F32R, tag="k_sb",
                                        name=f"k_sb_{g}_{t}")
                    nc.scalar.copy(k_sb[:], ktp[:])
                    vtp = pproj.tile([P, P], F32R, tag="proj",
                                     name=f"vtp_{g}_{t}")
                    nc.tensor.transpose(vtp[:], big["v", g][:, sl], ident[:])
                    v_sb = kv_pool.tile([P, P], F32R, tag="v_sb",
                                        name=f"v_sb_{g}_{t}")
                    nc.vector.tensor_copy(v_sb[:], vtp[:])
                    if pending[g] is not None:
                        p = pending[g]
                        nc.tensor.matmul(
                            m_psum[g][:], p[0][:], p[1][:],
                            start=(p[2] == 0), stop=(p[2] == TC - 1),
                            skip_group_check=True)
                    pending[g] = (k_sb, v_sb, t)

            def proj_group(tn, wn, bn, g, j, filler=0):
                ps = pproj.tile([P, SC], F32, tag="proj",
                                name=f"ps_{tn}{g}_{j}")
                for i in range(DC):
                    nc.tensor.matmul(
                        ps[:], w_tiles[wn, g][:, i, :], xs[i, j],
                        start=(i == 0), stop=(i == DC - 1),
                        skip_group_check=bool(filler),
                    )
                    # dummy matmuls between the accumulation steps: during
                    # the first chunk the x tiles land one DMA (~650ns)
                    # apart, slower than the PE consumes them, so the PE is
                    # provably idle here; the filler keeps the HAM clock
                    # gate open and soaks up the wait.
                    if filler and i < DC - 1:
                        for _ in range(filler):
                            nc.tensor.matmul(warm_ps[:], warm_in[:, 0:P],
                                             warm_in[:], start=True, stop=True,
                                             skip_group_check=True)
                sl = big[tn, g][:, j * SC:(j + 1) * SC]
                if tn == "v":
                    nc.scalar.activation(
                        sl, ps[:], mybir.ActivationFunctionType.Identity,
                        bias=b_tiles[bn, g][:])
                else:
                    nc.vector.tensor_scalar_add(sl, ps[:], b_tiles[bn, g][:])

            # j-outer, tensor-outer, g-inner: each x chunk feeds BOTH groups'
            # projections (48 matmuls per 2MB of DMA), so the HBM-bound x
            # stream never starves the PE after the first chunk, and each
            # weight tensor is first needed in the order the two DMA queues
            # deliver them (wq0 | wq1, wk1, wv1 (ACT) | wk0, wv0 (sync)).
            # Transposes trail one chunk behind.
            for g in range(NG):
                m_psum[g] = pm.tile([P, P], F32, tag="m", name=f"mps_{g}")
            for j in range(NSC):
                for g in range(NG):
                    for tn, wn, bn in (("q", "wq", "bq"), ("k", "wk", "bk"),
                                       ("v", "wv", "bv")):
                        proj_group(tn, wn, bn, g, j,
                                   filler=3 if (j == 0 and g == 0 and tn == "q")
                                   else 0)
                for g in range(NG):
                    if j > 0:
                        transpose_chunk(g, j - 1)
            for g in range(NG):
                transpose_chunk(g, NSC - 1)
                p = pending[g]
                nc.tensor.matmul(m_psum[g][:], p[0][:], p[1][:],
                                 start=(p[2] == 0), stop=True,
                                 skip_group_check=True)
                mt = mt_pool.tile([P, P], F32R, tag=f"mt{g}", name=f"mt_{g}")
                # zero-fill without InstMemset (walrus rejects f32r memset)
                nc.vector.tensor_scalar_mul(mt[:], ident[:], 0.0)
                nc.vector.tensor_copy(mt[0:E, 0:E], m_psum[g][0:E, 0:E])
                nc.vector.tensor_copy(mt[E:P, E:P], m_psum[g][E:P, E:P])

                # phase 3 for this group
                for j in range(NSC):
                    sl = slice(j * SC, (j + 1) * SC)
                    ps = pproj.tile([P, SC], F32, tag="proj", name=f"ops_{g}_{j}")
                    nc.tensor.matmul(ps[:], mt[:], big["q", g][:, sl],
                                     start=True, stop=True)
                    ot = ot_pool.tile([P, SC], F32, tag="ot", name=f"ot_{g}_{j}")
                    if j % 2 == 0:
                        nc.vector.tensor_copy(ot[:], ps[:])
                    else:
                        nc.scalar.copy(ot[:], ps[:])
                    nc.sync.dma_start(out_d[g, :, sl], ot[:])

    nc.compile()
    return nc


def _prep_inputs(x, Wq, Wk, Wv, bq, bk, bv):
    """Host-side shard + layout prep. Returns per-core input maps."""
    x_tiles_b = []
    for b in range(B):
        xf = np.ascontiguousarray(x[b].T)                   # [D, S]
        x_tiles_b.append(np.ascontiguousarray(
            xf.reshape(DC, P, NSC, SC).transpose(0, 2, 1, 3)))

    def wlayout(w):                                         # [P, D] -> [P, DC, P]
        return np.ascontiguousarray(w.T.reshape(DC, P, P).transpose(1, 0, 2))

    in_maps = []
    for c in range(NCORES):
        b = c // NB
        q0 = HL * (c % NB)                                  # first head of core
        m = {"x": x_tiles_b[b], "ident": np.eye(P, dtype=np.float32)}
        for g in range(NG):
            hs = slice(q0 + 2 * g, q0 + 2 * g + 2)
            m[f"wq{g}"] = wlayout((Wq[hs].reshape(P, D) * NORM).astype(np.float32))
            m[f"wk{g}"] = wlayout(Wk[hs].reshape(P, D).astype(np.float32))
            m[f"wv{g}"] = wlayout(Wv[hs].reshape(P, D).astype(np.float32))
            m[f"bq{g}"] = (bq[hs].reshape(P, 1) * NORM).astype(np.float32)
            m[f"bk{g}"] = bk[hs].reshape(P, 1).astype(np.float32)
            m[f"bv{g}"] = bv[hs].reshape(P, 1).astype(np.float32)
        in_maps.append(m)
    return in_maps


def _gather(results):
    out = np.empty((B, S, D), dtype=np.float32)
    for c in range(NCORES):
        b = c // NB
        oc = results[c]["outT"]                             # [NG, P, S]
        for g in range(NG):
            f0 = (c % NB) * (HL * E) + g * P
            out[b, :, f0:f0 + P] = oc[g].T
    return out


def get_compiled():
    global _compiled
    if _compiled is None:
        _compiled = _build()
    return _compiled


def run(in_maps):
    nc = get_compiled()
    return bass2jax.run_bass_via_pjrt(nc, in_maps, n_cores=NCORES)


def kernel(x, Wq, Wk, Wv, bq, bk, bv):
    in_maps = _prep_inputs(
        np.asarray(x, np.float32), np.asarray(Wq, np.float32),
        np.asarray(Wk, np.float32), np.asarray(Wv, np.float32),
        np.asarray(bq, np.float32), np.asarray(bk, np.float32),
        np.asarray(bv, np.float32),
    )
    return _gather(run(in_maps))


# revision 41
# speedup vs baseline: 1.0892x; 1.0131x over previous
"""Multi-head attention (no softmax) on 8 TRN2 NeuronCores.

Problem: x[2,2048,1024], per-head Wq/Wk/Wv[16,64,1024] + biases.
    q = einsum('bsd,hed->bhse', x, Wq) + bq   (same for k, v)
    out = ((q @ k^T) * E^-0.5) @ v, heads concatenated on feature dim.

Key algebraic fact: there is NO softmax, so
    (q k^T * norm) v = q @ (norm * (k^T v))
which collapses the O(S^2) attention into a 64x64 (per head) matmul.

Sharding: 2D tensor-parallel over (batch, head-quad): core c owns batch
c//4 and heads 4*(c%4) .. 4*(c%4)+3.  Each core reads only its batch's
half of x (8.4MB), and processes its 4 heads as two packed head-pairs
(feature groups g=0,1 of 128).

Per core (all matmuls fp32r = fp32 storage, ~fp22 multiply, fp32 psum):
  phase 1: project QT/KT/VT[g] = W[g] @ x_b^T in [feat(128), seq(2048)]
           layout (N=512 moving dim -> full PE speed).  norm is folded
           into Wq/bq on the host; biases are per-partition adds fused
           into the PSUM->SBUF copies.  j-outer / g-inner so each DMA'd
           x chunk feeds both groups (the x stream is HBM-bound).
  phase 2: trailing one seq-chunk behind phase 1: PE-transpose K/V tiles
           to [seq, feat] and accumulate M_g = K^T V [128,128] in PSUM
           over 16 seq-chunks (M matmul software-pipelined one step
           behind its transposes); copy the two diagonal 64x64 head
           blocks into a zeroed SBUF tile (cross-head blocks of M are
           garbage and must be dropped).
  phase 3: outT[g][:, s-chunk] = M_g(blockdiag).T @ QT[g][:, s-chunk].
Host gathers: out[c//4, s, (c%4)*256+g*128 +:128] = outT_c[g][:, s].T

Scheduling notes (why the DMA/program order looks the way it does):
  - one HWDGE dispatch is ~650ns regardless of size; the Sync queue
    carries only wq0-chunks interleaved with x(:,0), then wk0/wv0, then
    the rest of x.  Ident, biases and g=1 weights ride the ACT HWDGE
    queue (needed later; keeping them off the x stream matters).
  - dummy bf16 matmuls at kernel start hold the PE's HAM clock gate
    open (cold PE runs at 1.2GHz for its first ~3.4us) while the first
    x tiles land.
"""

import numpy as np

import concourse.bacc as bacc
import concourse.tile as tile
import concourse.mybir as mybir
from concourse import bass2jax

B, S, D, H = 2, 2048, 1024, 16
E = 64          # head dim
NCORES = 8
NB = NCORES // B            # cores per batch (4)
HL = H // NB                # heads per core (4)
NG = 2                      # feature groups per core (head pairs)
EP = HL * E // NG           # packed feature dim per group (128)
P = 128                     # partitions
DC = D // P                 # d chunks (8)
SC = 512                    # seq chunk for N=512 matmuls
NSC = S // SC               # 4 seq chunks
TC = S // P                 # 16 transpose chunks per group
NORM = float(E) ** -0.5

F32 = mybir.dt.float32
F32R = mybir.dt.float32r

_compiled = None


def _build():
    nc = bacc.Bacc("TRN2", target_bir_lowering=False, debug=False)

    x_d = nc.dram_tensor("x", [DC, NSC, P, SC], F32R, kind="ExternalInput").ap()
    w_d = {}
    for g in range(NG):
        for wn in ("wq", "wk", "wv"):
            w_d[wn, g] = nc.dram_tensor(
                f"{wn}{g}", [P, DC, P], F32R, kind="ExternalInput").ap()
    b_d = {}
    for g in range(NG):
        for bn in ("bq", "bk", "bv"):
            b_d[bn, g] = nc.dram_tensor(
                f"{bn}{g}", [P, 1], F32, kind="ExternalInput").ap()
    id_d = nc.dram_tensor("ident", [P, P], F32R, kind="ExternalInput").ap()
    out_d = nc.dram_tensor("outT", [NG, P, S], F32, kind="ExternalOutput").ap()

    with tile.TileContext(nc) as tc:
        with (
            tc.tile_pool(name="consts", bufs=1) as consts,
            tc.tile_pool(name="xs", bufs=32) as xs_pool,
            tc.tile_pool(name="qkv", bufs=1) as qkv_pool,
            tc.tile_pool(name="kv", bufs=8) as kv_pool,
            tc.tile_pool(name="mt", bufs=1) as mt_pool,
            tc.tile_pool(name="ot", bufs=8) as ot_pool,
            tc.tile_pool(name="pproj", bufs=6, space="PSUM") as pproj,
            tc.tile_pool(name="pm", bufs=2, space="PSUM") as pm,
        ):
            # ---- input DMAs (order matters; see scheduling notes above)
            xs = {}

            def load_x(i, j):
                t = xs_pool.tile([P, SC], F32R, tag="xs", name=f"x_{i}_{j}", bufs=16)
                nc.sync.dma_start(t[:], x_d[i, j])
                xs[i, j] = t[:]

            def load_x_pair(i):
                # j=2,3 ride one 512KB DMA: the HWDGE queue costs ~650ns
                # dispatch per DMA regardless of size, so the back half of
                # the x stream finishes ~5us earlier as 8 paired transfers
                t = xs_pool.tile([P, 2, SC], F32R, tag="xs2", name=f"x2_{i}", bufs=8)
                nc.sync.dma_start(t[:], x_d[i, 2:4].rearrange("a p s -> p a s"))
                xs[i, 2] = t[:, 0, :]
                xs[i, 3] = t[:, 1, :]

            w_tiles, b_tiles = {}, {}

            def load_w(wn, g):
                wt = consts.tile([P, DC, P], F32R, tag=f"{wn}{g}", name=f"{wn}{g}_t")
                nc.sync.dma_start(wt[:], w_d[wn, g][:])
                w_tiles[wn, g] = wt

            # interleave the first weight chunks with the first x tiles so
            # the first accumulation group can start after ~0.5MB of DMA
            wq0 = consts.tile([P, DC, P], F32R, tag="wq0", name="wq0_t")
            w_tiles["wq", 0] = wq0
            nc.sync.dma_start(wq0[:, 0:4, :], w_d["wq", 0][:, 0:4, :])
            load_x(0, 0)
            nc.sync.dma_start(wq0[:, 4:8, :], w_d["wq", 0][:, 4:8, :])
            for i in range(1, DC):
                load_x(i, 0)
            load_w("wk", 0)
            load_w("wv", 0)
            # Biases, ident and g=1 weights ride the ACT HWDGE queue so the
            # Sync queue stays a pure x stream (it is HBM-bound; every DMA
            # inserted into it delays the x tiles phase 1 is waiting for).
            # Order: ident + g=1 weights are needed within ~10-16us (the
            # j-interleaved schedule uses both groups' weights early).
            ident = consts.tile([P, P], F32R, tag="ident")
            nc.scalar.dma_start(ident[:], id_d[:])
            for wn in ("wq", "wk", "wv"):
                wt = consts.tile([P, DC, P], F32R, tag=f"{wn}1", name=f"{wn}1_t")
                nc.scalar.dma_start(wt[:], w_d[wn, 1][:])
                w_tiles[wn, 1] = wt
            for bn in ("bq", "bk", "bv"):
                for g in range(NG):
                    bt = consts.tile([P, 1], F32, tag=f"{bn}{g}", name=f"{bn}{g}_t")
                    nc.scalar.dma_start(bt[:], b_d[bn, g][:])
                    b_tiles[bn, g] = bt
            for i in range(DC):
                load_x(i, 1)
            for i in range(DC):
                load_x_pair(i)

            # ---- PE clock warmup: the HAM gate holds the PE at 1.2GHz for
            # its first ~3.4us of sustained activity, and the PE is idle
            # until the first x tiles land anyway.  Burn that window with
            # dummy matmuls on a zeroed scratch tile so the real matmuls
            # start at 2.4GHz.  Results go to a psum tile nobody reads.
            warm_in = consts.tile([P, 256], mybir.dt.bfloat16, tag="warm")
            nc.gpsimd.memset(warm_in[:], 0.0)
            warm_ps = pproj.tile([P, 256], F32, tag="proj", name="warm_ps")
            for _ in range(10):
                nc.tensor.matmul(warm_ps[:], warm_in[:, 0:P], warm_in[:],
                                 start=True, stop=True)

            # ---- phases 1+2, transposes trailing one seq chunk
            big = {}
            for g in range(NG):
                for tn in ("q", "k", "v"):
                    big[tn, g] = qkv_pool.tile([P, S], F32R, tag=f"{tn}t{g}",
                                               name=f"{tn}t{g}")
            m_psum = {}
            pending = {g: None for g in range(NG)}  # per-g pipelined M matmul

            def transpose_chunk(g, j):
                for tt in range(SC // P):
                    t = j * (SC // P) + tt
                    sl = slice(t * P, (t + 1) * P)
                    ktp = pproj.tile([P, P], F32R, tag="proj",
                                     name=f"ktp_{g}_{t}")
                    nc.tensor.transpose(ktp[:], big["k", g][:, sl], ident[:])
                    k_sb = kv_pool.tile([P, P], F32R, tag="k_sb",
                                        name=f"k_sb_{g}_{t}")
                    nc.scalar.copy(k_sb[:], ktp[:])
                    vtp = pproj.tile([P, P], F32R, tag="proj",
                                     name=f"vtp_{g}_{t}")
                    nc.tensor.transpose(vtp[:], big["v", g][:, sl], ident[:])
                    v_sb = kv_pool.tile([P, P], F32R, tag="v_sb",
                                        name=f"v_sb_{g}_{t}")
                    nc.vector.tensor_copy(v_sb[:], vtp[:])
                    if pending[g] is not None:
                        p = pending[g]
                        nc.tensor.matmul(
                            m_psum[g][:], p[0][:], p[1][:],
                            start=(p[2] == 0), stop=(p[2] == TC - 1),
                            skip_group_check=True)
                    pending[g] = (k_sb, v_sb, t)

            def proj_group(tn, wn, bn, g, j, filler=0):
                ps = pproj.tile([P, SC], F32, tag="proj",
                                name=f"ps_{tn}{g}_{j}")
                for i in range(DC):
                    nc.tensor.matmul(
                        ps[:], w_tiles[wn, g][:, i, :], xs[i, j],
                        start=(i == 0), stop=(i == DC - 1),
                        skip_group_check=bool(filler),
                    )
                    # dummy matmuls between the accumulation steps: during
                    # the first chunk the x tiles land one DMA (~650ns)
                    # apart, slower than the PE consumes them, so the PE is
                    # provably idle here; the filler keeps the HAM clock
                    # gate open and soaks up the wait.
                    if filler and i < DC - 1:
                        for _ in range(filler):
                            nc.tensor.matmul(warm_ps[:], warm_in[:, 0:P],
                                             warm_in[:], start=True, stop=True,
                                             skip_group_check=True)
                sl = big[tn, g][:, j * SC:(j + 1) * SC]
                if tn == "v":
                    nc.scalar.activation(
                        sl, ps[:], mybir.ActivationFunctionType.Identity,
                        bias=b_tiles[bn, g][:])
                else:
                    nc.vector.tensor_scalar_add(sl, ps[:], b_tiles[bn, g][:])

            # j-outer, tensor-outer, g-inner: each x chunk feeds BOTH groups'
            # projections (48 matmuls per 2MB of DMA), so the HBM-bound x
            # stream never starves the PE after the first chunk, and each
            # weight tensor is first needed in the order the two DMA queues
            # deliver them (wq0 | wq1, wk1, wv1 (ACT) | wk0, wv0 (sync)).
            # Transposes trail one chunk behind.
            for g in range(NG):
                m_psum[g] = pm.tile([P, P], F32, tag="m", name=f"mps_{g}")
            for j in range(NSC):
                for g in range(NG):
                    for tn, wn, bn in (("q", "wq", "bq"), ("k", "wk", "bk"),
                                       ("v", "wv", "bv")):
                        fl = 0
                        if j == 0 and g == 0:
                            fl = {"q": 5, "k": 2, "v": 0}[tn]
                        proj_group(tn, wn, bn, g, j, filler=fl)
                for g in range(NG):
                    if j > 0:
                        transpose_chunk(g, j - 1)
            for g in range(NG):
                transpose_chunk(g, NSC - 1)
                p = pending[g]
                nc.tensor.matmul(m_psum[g][:], p[0][:], p[1][:],
                                 start=(p[2] == 0), stop=True,
                                 skip_group_check=True)
                mt = mt_pool.tile([P, P], F32R, tag=f"mt{g}", name=f"mt_{g}")
                # zero-fill without InstMemset (walrus rejects f32r memset)
                nc.vector.tensor_scalar_mul(mt[:], ident[:], 0.0)
                nc.vector.tensor_copy(mt[0:E, 0:E], m_psum[g][0:E, 0:E])
                nc.vector.tensor_copy(mt[E:P, E:P], m_psum[g][E:P, E:P])

                # phase 3 for this group
                for j in range(NSC):
                    sl = slice(j * SC, (j + 1) * SC)
                    ps = pproj.tile([P, SC], F32, tag="proj", name=f"ops_{g}_{j}")
                    nc.tensor.matmul(ps[:], mt[:], big["q", g][:, sl],
                                     start=True, stop=True)
                    ot = ot_pool.tile([P, SC], F32, tag="ot", name=f"ot_{g}_{j}")
                    if j % 2 == 0:
                        nc.vector.tensor_copy(ot[:], ps[:])
                    else:
                        nc.scalar.copy(ot[:], ps[:])
                    nc.sync.dma_start(out_d[g, :, sl], ot[:])

    nc.compile()
    return nc


def _prep_inputs(x, Wq, Wk, Wv, bq, bk, bv):
    """Host-side shard + layout prep. Returns per-core input maps."""
    x_tiles_b = []
    for b in range(B):
        xf = np.ascontiguousarray(x[b].T)                   # [D, S]
        x_tiles_b.append(np.ascontiguousarray(
            xf.reshape(DC, P, NSC, SC).transpose(0, 2, 1, 3)))

    def wlayout(w):                                         # [P, D] -> [P, DC, P]
        return np.ascontiguousarray(w.T.reshape(DC, P, P).transpose(1, 0, 2))

    in_maps = []
    for c in range(NCORES):
        b = c // NB
        q0 = HL * (c % NB)                                  # first head of core
        m = {"x": x_tiles_b[b], "ident": np.eye(P, dtype=np.float32)}
        for g in range(NG):
            hs = slice(q0 + 2 * g, q0 + 2 * g + 2)
            m[f"wq{g}"] = wlayout((Wq[hs].reshape(P, D) * NORM).astype(np.float32))
            m[f"wk{g}"] = wlayout(Wk[hs].reshape(P, D).astype(np.float32))
            m[f"wv{g}"] = wlayout(Wv[hs].reshape(P, D).astype(np.float32))
            m[f"bq{g}"] = (bq[hs].reshape(P, 1) * NORM).astype(np.float32)
            m[f"bk{g}"] = bk[hs].reshape(P, 1).astype(np.float32)
            m[f"bv{g}"] = bv[hs].reshape(P, 1).astype(np.float32)
        in_maps.append(m)
    return in_maps


def _gather(results):
    out = np.empty((B, S, D), dtype=np.float32)
    for c in range(NCORES):
        b = c // NB
        oc = results[c]["outT"]                             # [NG, P, S]
        for g in range(NG):
            f0 = (c % NB) * (HL * E) + g * P
            out[b, :, f0:f0 + P] = oc[g].T
    return out


def get_compiled():
    global _compiled
    if _compiled is None:
        _compiled = _build()
    return _compiled


def run(in_maps):
    nc = get_compiled()
    return bass2jax.run_bass_via_pjrt(nc, in_maps, n_cores=NCORES)


def kernel(x, Wq, Wk, Wv, bq, bk, bv):
    in_maps = _prep_inputs(
        np.asarray(x, np.float32), np.asarray(Wq, np.float32),
        np.asarray(Wk, np.float32), np.asarray(Wv, np.float32),
        np.asarray(bq, np.float32), np.asarray(bk, np.float32),
        np.asarray(bv, np.float32),
    )
    return _gather(run(in_maps))
